# revision 18
# baseline (speedup 1.0000x reference)
"""Trainium2 Bass kernel for 2-layer bipartite GNN propagation (MDCLBR).

Design (v3):
- Dest rows of each graph are dealt round-robin across the 8 cores
  (side-blocked: A=users then B=items/bundles, each side padded to a tile
  boundary), so every core sees a statistically identical workload and the
  SPMD max-over-cores chunk padding is small.
- Source feature tables are stored TRANSPOSED ([feat, node] fp32) and kept
  resident in SBUF, split in two bucket-halves on partition halves 0-63 /
  64-127.  Edges are gathered with the gpsimd compute gather (ap_gather):
  partition f receives feature f of each edge's source node.  The two
  partition halves gather two independent chunk streams (one per bucket).
- Per 128-edge chunk: PE transposes the gathered [64,128] block into PSUM,
  ACT copies it to SBUF as bf16, DVE builds a scaled one-hot selection
  matrix (iota is_equal row * val, bf16), and the PE accumulates the
  segment sum in PSUM via matmul.
- Per dest tile: ACT scales/copies PSUM, computes the squared-row-norm with
  an accumulating Square and Rsqrt, DVE updates the layer accumulator.
  Layer-1 features (and the item accumulator for the BI aggregation) are
  written back transposed; AllGather assembles the next layer's tables.
"""
import sys
sys.path.insert(0, '/opt/trn_rl_repo')
import numpy as np
import ml_dtypes

U, I, B, D = 50000, 40000, 20000, 64
NC = 8
BF16 = ml_dtypes.bfloat16


def _pad_tiles(per):
    return -(-per // 128) * 128


# side-block geometry (slots per core)
A_PER, A_PAD = U // NC, _pad_tiles(U // NC)          # 6250, 6272
IB_PER, IB_PAD = I // NC, _pad_tiles(I // NC)        # 5000, 5120
BB_PER, BB_PAD = B // NC, _pad_tiles(B // NC)        # 2500, 2560
IL_ROWS = A_PAD + IB_PAD                             # 11392
BL_ROWS = A_PAD + BB_PAD                             # 8832
BI_ROWS = BB_PAD                                     # 2560

_compiled = None


def _deal(vec, per, pad):
    """global side-row -> (core, slot)"""
    return vec % NC, vec // NC


def _perm_order(n, per, pad):
    """host-side: permuted table row p = core*pad + slot -> global row, and
    inverse map global -> table row"""
    g = np.arange(n)
    tab = (g % NC) * pad + g // NC
    return tab


class _Phase:
    """One (spmm, dest-side) phase: tiles [t0, t1) of the spmm's local tile
    space, gathering from a 2-half bucket table."""
    __slots__ = ('t0', 't1', 'half_span', 'supers', 'C', 'name', 'base_chunk')


def _layout_phase(core, slot, trow, vals, t0, t1, half_span, sup_tiles, name):
    """Chunk layout for one phase.

    core/slot: dest (core, slot) per edge; trow: source table row (dealt
    order) per edge; half h = trow >= half_span.
    Returns phase descriptor + per-edge placement arrays:
      chunk id (global within phase, 0..C), partition, idx value.
    """
    t = slot // 128
    h = (trow >= half_span).astype(np.int64)
    idxv = trow - h * half_span
    T = t1 - t0
    key = ((core * T + (t - t0)) * 2 + h)
    counts = np.bincount(key, minlength=NC * T * 2).reshape(NC, T, 2)
    K = -(-counts.max(axis=0) // 128)              # [T, 2]
    # supers: group tiles; per super, two streams (h0, h1) padded to equal
    # chunk count
    ph = _Phase()
    ph.t0, ph.t1, ph.half_span, ph.name = t0, t1, half_span, name
    supers = []
    choff = 0           # global chunk counter (phase-local)
    chunk_of_block = {}
    for s0 in range(0, T, sup_tiles):
        ts = list(range(s0, min(s0 + sup_tiles, T)))
        k0 = int(K[ts, 0].sum())
        k1 = int(K[ts, 1].sum())
        nk = max(k0, k1, 1)
        sup = {'nk': nk, 'tiles': [], 'choff': choff}
        # stream positions: h stream chunk j -> gather col j
        pos = [0, 0]
        for tt in ts:
            tb = []
            for h_ in (0, 1):
                kk = int(K[tt, h_])
                if kk:
                    tb.append((h_, kk, pos[h_]))
                    chunk_of_block[(tt, h_)] = (choff, pos[h_])
                    pos[h_] += kk
            sup['tiles'].append((tt + t0, tb))
        supers.append(sup)
        choff += nk
    ph.supers = supers
    ph.C = choff
    # per-edge placement
    order = np.argsort(key, kind='stable')
    skey = key[order]
    gstart = np.zeros(NC * T * 2, np.int64)
    np.cumsum(counts.reshape(-1)[:-1], out=gstart[1:])
    within = np.arange(len(order)) - gstart[skey]
    so_t = (t - t0)[order]
    so_h = h[order]
    # chunk position within the super's stream
    bo = np.array([chunk_of_block.get((tt, hh), (0, 0))
                   for tt, hh in zip(so_t, so_h)])
    sup_choff = bo[:, 0] if len(bo) else np.zeros(0, np.int64)
    stream_pos = bo[:, 1] if len(bo) else np.zeros(0, np.int64)
    chunk = sup_choff + stream_pos + within // 128
    part = within % 128
    return ph, order, so_h, chunk, part, idxv[order], vals[order], \
        (slot % 128)[order]


class _Graph:
    __slots__ = ('name', 'nc_rows', 'phases', 'idx16', 'rows_f', 'vals_f',
                 'Cidx', 'C')


def _build_graph(name, rows_core, rows_slot, trow, vals, nc_rows, phase_specs):
    """phase_specs: list of (t0, t1, half_span, sup_tiles, edge_mask, pname)"""
    g = _Graph()
    g.name, g.nc_rows = name, nc_rows
    g.phases = []
    placements = []
    C = 0
    for (t0, t1, half_span, sup_tiles, mask, pname) in phase_specs:
        ph, order, so_h, chunk, part, idxv, v, r128 = _layout_phase(
            rows_core[mask], rows_slot[mask], trow[mask], vals[mask],
            t0, t1, half_span, sup_tiles, pname)
        ph.base_chunk = C
        placements.append((rows_core[mask][order], so_h, chunk + C, part,
                           idxv, v, r128))
        # shift super choffs to global chunk ids
        for s in ph.supers:
            s['choff'] += C
        C += ph.C
        g.phases.append(ph)
    g.C = C
    # idx table: [NC, 128, C*8] int16 (per 16-part group wrapped, stream h in
    # partition halves, replicated x4 within half)
    idx16 = np.zeros((NC, 128, C * 8), np.int16)
    rows_f = np.zeros((NC, 128, 2 * C), np.float32)
    vals_f = np.zeros((NC, 128, 2 * C), np.float32)
    for (ecore, eh, echunk, epart, eidx, ev, er) in placements:
        rows_f[ecore, epart, 2 * echunk + eh] = er.astype(np.float32)
        vals_f[ecore, epart, 2 * echunk + eh] = ev.astype(np.float32)
        col = echunk * 8 + epart // 16
        prow = (epart % 16) + (eh * 64)
        for rep in range(4):
            idx16[ecore, rep * 16 + prow, col] = eidx.astype(np.int16)
    g.idx16, g.rows_f, g.vals_f = idx16, rows_f, vals_f
    return g


def _prep(inputs):
    il_rows = np.asarray(inputs['il_rows']).astype(np.int64)
    il_cols = np.asarray(inputs['il_cols']).astype(np.int64)
    il_vals = np.asarray(inputs['il_vals']).astype(np.float32)
    bl_rows = np.asarray(inputs['bl_rows']).astype(np.int64)
    bl_cols = np.asarray(inputs['bl_cols']).astype(np.int64)
    bl_vals = np.asarray(inputs['bl_vals']).astype(np.float32)
    bi_rows = np.asarray(inputs['bi_rows']).astype(np.int64)
    bi_cols = np.asarray(inputs['bi_cols']).astype(np.int64)
    bi_vals = np.asarray(inputs['bi_vals']).astype(np.float32)

    def dest_map(r, nA, padA, padB):
        isB = r >= nA
        q = np.where(isB, r - nA, r)
        return q % NC, np.where(isB, padA + q // NC, q // NC)

    def src_map(c, nA, padA_t, padB_t):
        """source table row in side-dealt order; A table rows [0, NC*padA_t),
        B table rows offset 0 in their own table."""
        isB = c >= nA
        q = np.where(isB, c - nA, c)
        return isB, (q % NC) * np.where(isB, padB_t, padA_t) + q // NC

    # ---- il ----
    core, slot = dest_map(il_rows, U, A_PAD, IB_PAD)
    isB, trow = src_map(il_cols, U, A_PAD, IB_PAD)
    TA, TB = A_PAD // 128, IB_PAD // 128     # 49, 40
    # phase alpha: dest-B tiles (cols are A-side, table NTA = NC*A_PAD=50176)
    # phase beta: dest-A tiles (cols B-side, table NC*IB_PAD = 40960)
    NTA, NTB_il = NC * A_PAD, NC * IB_PAD
    g_il = _build_graph(
        'il', core, slot, np.where(isB, trow, trow), il_vals, IL_ROWS,
        [(TA, TA + TB, NTA // 2, 2, ~isB, 'a'),      # dest-B <- src A
         (0, TA, NTB_il // 2, 3, isB, 'b')])         # dest-A <- src B

    # ---- bl ----
    core, slot = dest_map(bl_rows, U, A_PAD, BB_PAD)
    isB, trow = src_map(bl_cols, U, A_PAD, BB_PAD)
    TBB = BB_PAD // 128                      # 20
    NTB_bl = NC * BB_PAD                     # 20480
    g_bl = _build_graph(
        'bl', core, slot, trow, bl_vals, BL_ROWS,
        [(TA, TA + TBB, NTA // 2, 4, ~isB, 'a'),
         (0, TA, NTB_bl // 2, 4, isB, 'b')])

    # ---- bi ---- dest bundles (side-A of its own space), src = il item acc
    core, slot = bi_rows % NC, bi_rows // NC
    trow = (bi_cols % NC) * IB_PAD + bi_cols // NC
    g_bi = _build_graph(
        'bi', core, slot, trow, bi_vals, BI_ROWS,
        [(0, TBB, NTB_il // 2, 2, np.ones(len(bi_rows), bool), 'a')])

    return g_il, g_bl, g_bi


def _build_program(g_il, g_bl, g_bi):
    from concourse import mybir, bacc
    import concourse.tile as tile

    f32, bf16, i16 = mybir.dt.float32, mybir.dt.bfloat16, mybir.dt.int16
    ACT = mybir.ActivationFunctionType
    nc = bacc.Bacc("TRN2", target_bir_lowering=False, debug=False,
                   num_devices=NC)

    NTA = NC * A_PAD          # 50176 user table rows
    NTB_il = NC * IB_PAD      # 40960
    NTB_bl = NC * BB_PAD      # 20480

    # host-provided transposed fp32 tables + misc
    xA_T = nc.dram_tensor("xA_T", [64, NTA, 1], f32, kind="ExternalInput")
    xBi_T = nc.dram_tensor("xBi_T", [64, NTB_il, 1], f32, kind="ExternalInput")
    xBb_T = nc.dram_tensor("xBb_T", [64, NTB_bl, 1], f32, kind="ExternalInput")
    x0_il = nc.dram_tensor("x0_il", [IL_ROWS, D], f32, kind="ExternalInput")
    x0_bl = nc.dram_tensor("x0_bl", [BL_ROWS, D], f32, kind="ExternalInput")
    iota_d = nc.dram_tensor("iota_d", [128, 128], bf16, kind="ExternalInput")
    ident_d = nc.dram_tensor("ident_d", [128, 128], f32, kind="ExternalInput")
    ins_t = {}
    for g in (g_il, g_bl, g_bi):
        ins_t[g.name] = (
            nc.dram_tensor(f"{g.name}_idx", [128, g.C * 8], i16,
                           kind="ExternalInput"),
            nc.dram_tensor(f"{g.name}_rows", [128, 2 * g.C], f32,
                           kind="ExternalInput"),
            nc.dram_tensor(f"{g.name}_vals", [128, 2 * g.C], f32,
                           kind="ExternalInput"),
        )
    il_acc_mid = nc.dram_tensor("il_acc_mid", [IL_ROWS, D], f32)
    bl_acc_mid = nc.dram_tensor("bl_acc_mid", [BL_ROWS, D], f32)
    il_acc_out = nc.dram_tensor("il_acc_out", [IL_ROWS, D], f32,
                                kind="ExternalOutput")
    bl_acc_out = nc.dram_tensor("bl_acc_out", [BL_ROWS, D], f32,
                                kind="ExternalOutput")
    bi_out = nc.dram_tensor("bi_out", [BI_ROWS, D], f32,
                            kind="ExternalOutput")

    # AG staging: transposed f1 slices and item-acc slices
    il_f1T = nc.dram_tensor("il_f1T", [64, IL_ROWS], f32)
    il_f1T_full = nc.dram_tensor("il_f1T_full", [64 * NC, IL_ROWS], f32,
                                 addr_space="Shared")
    bl_f1T = nc.dram_tensor("bl_f1T", [64, BL_ROWS], f32)
    bl_f1T_full = nc.dram_tensor("bl_f1T_full", [64 * NC, BL_ROWS], f32,
                                 addr_space="Shared")
    gbounce = nc.dram_tensor("gbounce", [64, 4 * 8192], f32)
    accT = nc.dram_tensor("accT", [64, IB_PAD], f32)
    accT_full = nc.dram_tensor("accT_full", [64 * NC, IB_PAD], f32,
                               addr_space="Shared")
    RG = [list(range(NC))]

    with tile.TileContext(nc) as tc:
        with (
            tc.tile_pool(name="const", bufs=1) as cpool,
            tc.tile_pool(name="tabs", bufs=1) as tabpool,
            tc.tile_pool(name="meta", bufs=2) as mpool,
            tc.tile_pool(name="idx", bufs=3) as ipool,
            tc.tile_pool(name="gath", bufs=2) as gpool,
            tc.tile_pool(name="gdn", bufs=2) as dpool,
            tc.tile_pool(name="sel", bufs=6) as spool,
            tc.tile_pool(name="gcast", bufs=6) as bpool,
            tc.tile_pool(name="ptr", bufs=4, space="PSUM") as trpool,
            tc.tile_pool(name="ptrf", bufs=2, space="PSUM") as trfpool,
            tc.tile_pool(name="pacc", bufs=2, space="PSUM") as ppool,
            tc.tile_pool(name="feats", bufs=3) as fpool,
            tc.tile_pool(name="nrm", bufs=4) as npool,
        ):
            iota_b = cpool.tile([128, 128], bf16)
            nc.sync.dma_start(iota_b[:], iota_d[:])
            ident = cpool.tile([128, 128], f32)
            nc.sync.dma_start(ident[:], ident_d[:])

            def load_table_host(src, span2):
                """load a host transposed table into [128, span, 1] tile"""
                t = tabpool.tile([128, span2, 1], f32, tag="tab")
                nc.scalar.dma_start(t[0:64, :, :], src[:, 0:span2, :])
                nc.scalar.dma_start(t[64:128, :, :], src[:, span2:2 * span2, :])
                return t

            def load_table_ag(agfull, nc_rows, c0, c1, span2):
                """assemble table from AG output [64*NC, nc_rows], cols
                [c0, c1) of each 64-row band -> [128, span2, 1] (2 halves)"""
                t = tabpool.tile([128, span2, 1], f32, tag="tab")
                w = c1 - c0
                for c in range(NC):
                    lo, hi = c * w, (c + 1) * w
                    for hb in (0, 1):
                        s0, s1 = hb * span2, (hb + 1) * span2
                        # intersect [lo,hi) with [s0,s1)
                        a, b = max(lo, s0), min(hi, s1)
                        if a < b:
                            nc.scalar.dma_start(
                                t[hb * 64:hb * 64 + 64, a - s0:b - s0, 0],
                                agfull[c * 64:(c + 1) * 64,
                                       c0 + (a - lo):c0 + (b - lo)])
                return t

            def spmm_phase(g, ph, tab, scale, layer, acc_mid, x0_dram,
                           f1T_dram, accT_dram, acc_out):
                idx_d, rows_d, vals_d = ins_t[g.name]
                cb = 2 * ph.base_chunk
                rows_sb = mpool.tile([128, 2 * ph.C], f32, tag="rows")
                vals_sb = mpool.tile([128, 2 * ph.C], f32, tag="vals")
                nc.scalar.dma_start(rows_sb[:], rows_d[:, cb:cb + 2 * ph.C])
                nc.scalar.dma_start(vals_sb[:], vals_d[:, cb:cb + 2 * ph.C])
                span2 = ph.half_span
                for si, sup in enumerate(ph.supers):
                    nk = sup['nk']
                    gbo = (si % 4) * 8192
                    ioff = sup['choff']
                    idx_t = ipool.tile([128, nk * 8], i16, tag="idx")
                    nc.scalar.dma_start(idx_t[:],
                                        idx_d[:, ioff * 8:(ioff + nk) * 8])
                    g_t = gpool.tile([128, nk * 128, 1], f32, tag="g")
                    nc.gpsimd.ap_gather(
                        out_ap=g_t[:], in_ap=tab[:],
                        idxs_ap=idx_t[:], channels=128, num_elems=span2,
                        d=1, num_idxs=nk * 128)
                    nk1 = sum(kk for _, tb_ in sup['tiles']
                              for hh, kk, _ in tb_ if hh == 1)
                    g_d = None
                    if nk1 > 0:
                        g_d = dpool.tile([64, nk * 128], f32, tag="gd")
                        nc.scalar.dma_start(gbounce[:, gbo:gbo + nk1 * 128],
                                            g_t[64:128, 0:nk1 * 128, 0])
                        nc.scalar.dma_start(g_d[:, 0:nk1 * 128],
                                            gbounce[:, gbo:gbo + nk1 * 128])
                    for tt, tb in sup['tiles']:
                        nchunks = sum(kk for _, kk, _ in tb)
                        if nchunks == 0:
                            continue
                        psum_t = ppool.tile([128, D], f32, tag="ps")
                        done = 0
                        for h_, kk, pos0 in tb:
                            base = sup['choff'] + pos0
                            for k in range(kk):
                                cid = 2 * (base + k) + h_
                                co = (pos0 + k) * 128
                                ptr = trpool.tile([128, 64], f32, tag="tr")
                                src_ap = (g_t[0:64, co:co + 128, 0]
                                          if h_ == 0 else
                                          g_d[:, co:co + 128])
                                nc.tensor.matmul(
                                    ptr[:], src_ap, ident[0:64, 0:64],
                                    is_transpose=True,
                                    skip_group_check=True)
                                g_b = bpool.tile([128, 64], bf16, tag="gb")
                                nc.scalar.activation(g_b[:], ptr[:], ACT.Copy)
                                s_t = spool.tile([128, 128], bf16, tag="s")
                                nc.vector.tensor_scalar(
                                    out=s_t[:], in0=iota_b[:],
                                    scalar1=rows_sb[:, cid - cb:cid - cb + 1],
                                    scalar2=vals_sb[:, cid - cb:cid - cb + 1],
                                    op0=mybir.AluOpType.is_equal,
                                    op1=mybir.AluOpType.mult)
                                nc.tensor.matmul(
                                    psum_t[:], s_t[:], g_b[:],
                                    start=(done == 0),
                                    stop=(done == nchunks - 1),
                                    skip_group_check=True)
                                done += 1
                        # ---- tile epilogue ----
                        if layer is None:
                            o_t = fpool.tile([128, D], f32, tag="f")
                            nc.scalar.activation(o_t[:], psum_t[:], ACT.Copy)
                            nc.sync.dma_start(
                                bi_out[tt * 128:(tt + 1) * 128, :], o_t[:])
                            continue
                        f_s = fpool.tile([128, D], f32, tag="f")
                        nc.scalar.activation(f_s[:], psum_t[:], ACT.Copy,
                                             scale=scale)
                        sq = npool.tile([128, D], f32, tag="sq")
                        n2 = npool.tile([128, 1], f32, tag="n2")
                        nc.scalar.activation(sq[:], f_s[:], ACT.Square,
                                             accum_out=n2[:])
                        nr = npool.tile([128, 1], f32, tag="nr")
                        nc.scalar.activation(nr[:], n2[:], ACT.Sqrt)
                        nc.vector.tensor_scalar_max(nr[:], nr[:], 1e-12)
                        ri = npool.tile([128, 1], f32, tag="ri")
                        nc.vector.reciprocal(ri[:], nr[:])
                        ao = fpool.tile([128, D], f32, tag="ao")
                        if layer == 0:
                            x0_t = fpool.tile([128, D], f32, tag="x0")
                            nc.sync.dma_start(
                                x0_t[:], x0_dram[tt * 128:(tt + 1) * 128, :])
                            nc.vector.scalar_tensor_tensor(
                                out=ao[:], in0=f_s[:], scalar=ri[:, 0:1],
                                in1=x0_t[:], op0=mybir.AluOpType.mult,
                                op1=mybir.AluOpType.add)
                            nc.sync.dma_start(
                                acc_mid[tt * 128:(tt + 1) * 128, :], ao[:])
                            # write f1 transposed for next layer's table
                            ptr2 = trfpool.tile([64, 128], f32, tag="trf")
                            nc.tensor.matmul(ptr2[:], f_s[:], ident[:],
                                             is_transpose=True,
                                             skip_group_check=True)
                            fT = fpool.tile([64, 128], f32, tag="fT")
                            nc.scalar.activation(fT[:], ptr2[:], ACT.Copy)
                            nc.sync.dma_start(
                                f1T_dram[:, tt * 128:(tt + 1) * 128],
                                fT[:])
                        else:
                            am = fpool.tile([128, D], f32, tag="am")
                            nc.sync.dma_start(
                                am[:], acc_mid[tt * 128:(tt + 1) * 128, :])
                            nc.vector.scalar_tensor_tensor(
                                out=ao[:], in0=f_s[:], scalar=ri[:, 0:1],
                                in1=am[:], op0=mybir.AluOpType.mult,
                                op1=mybir.AluOpType.add)
                            nc.sync.dma_start(
                                acc_out[tt * 128:(tt + 1) * 128, :],
                                ao[:])
                            if accT_dram is not None and tt >= g.phases[0].t0:
                                # item tiles: transposed acc for BI table
                                ptr2 = trfpool.tile([64, 128], f32, tag="trf")
                                nc.tensor.matmul(ptr2[:], ao[:], ident[:],
                                                 is_transpose=True,
                                                 skip_group_check=True)
                                fT = fpool.tile([64, 128], f32, tag="fT")
                                nc.scalar.activation(fT[:], ptr2[:], ACT.Copy)
                                t0 = g.phases[0].t0
                                nc.sync.dma_start(
                                    accT_dram[:, (tt - t0) * 128:
                                              (tt - t0 + 1) * 128],
                                    fT[:])

            TA = A_PAD // 128

            # P1: users table; il-L1-alpha (item tiles)
            tabA = load_table_host(xA_T, NTA // 2)
            spmm_phase(g_il, g_il.phases[0], tabA, 0.5, 0, il_acc_mid, x0_il,
                       il_f1T, None, il_acc_out)
            # P2: bl-L1-alpha (bundle tiles) -- same users table
            spmm_phase(g_bl, g_bl.phases[0], tabA, 0.5, 0, bl_acc_mid, x0_bl,
                       bl_f1T, None, bl_acc_out)
            # P3: items-x table; il-L1-beta (user tiles)
            tabBi = load_table_host(xBi_T, NTB_il // 2)
            spmm_phase(g_il, g_il.phases[1], tabBi, 0.5, 0, il_acc_mid, x0_il,
                       il_f1T, None, il_acc_out)
            nc.gpsimd.collective_compute(
                "AllGather", mybir.AluOpType.bypass, ins=[il_f1T[:]],
                outs=[il_f1T_full[:]], replica_groups=RG)
            # P4: bundles-x table; bl-L1-beta (user tiles)
            tabBb = load_table_host(xBb_T, NTB_bl // 2)
            spmm_phase(g_bl, g_bl.phases[1], tabBb, 0.5, 0, bl_acc_mid, x0_bl,
                       bl_f1T, None, bl_acc_out)
            nc.gpsimd.collective_compute(
                "AllGather", mybir.AluOpType.bypass, ins=[bl_f1T[:]],
                outs=[bl_f1T_full[:]], replica_groups=RG)
            # P5: il-f1 user table; il-L2-alpha (item tiles)
            tabf1A = load_table_ag(il_f1T_full, IL_ROWS, 0, A_PAD, NTA // 2)
            spmm_phase(g_il, g_il.phases[0], tabf1A, 1.0 / 3, 1, il_acc_mid,
                       None, None, accT, il_acc_out)
            nc.gpsimd.collective_compute(
                "AllGather", mybir.AluOpType.bypass, ins=[accT[:]],
                outs=[accT_full[:]], replica_groups=RG)
            # P6: il-f1 item table; il-L2-beta (user tiles)
            tabf1B = load_table_ag(il_f1T_full, IL_ROWS, A_PAD, IL_ROWS,
                                   NTB_il // 2)
            spmm_phase(g_il, g_il.phases[1], tabf1B, 1.0 / 3, 1, il_acc_mid,
                       None, None, None, il_acc_out)
            # P7: bl-f1 user table; bl-L2-alpha
            tabg1A = load_table_ag(bl_f1T_full, BL_ROWS, 0, A_PAD, NTA // 2)
            spmm_phase(g_bl, g_bl.phases[0], tabg1A, 1.0 / 3, 1, bl_acc_mid,
                       None, None, None, bl_acc_out)
            # P8: bl-f1 bundle table; bl-L2-beta
            tabg1B = load_table_ag(bl_f1T_full, BL_ROWS, A_PAD, BL_ROWS,
                                   NTB_bl // 2)
            spmm_phase(g_bl, g_bl.phases[1], tabg1B, 1.0 / 3, 1, bl_acc_mid,
                       None, None, None, bl_acc_out)
            # P9: bi aggregation from item acc
            tabacc = load_table_ag(accT_full, IB_PAD, 0, IB_PAD, NTB_il // 2)
            spmm_phase(g_bi, g_bi.phases[0], tabacc, 1.0, None, None, None,
                       None, None, None)

    nc.compile()
    return nc


def kernel(users_feature, items_feature, bundles_feature,
           il_rows, il_cols, il_vals,
           bl_rows, bl_cols, bl_vals,
           bi_rows, bi_cols, bi_vals):
    global _compiled
    from concourse.bass_utils import run_bass_kernel_spmd

    xu = np.asarray(users_feature, np.float32)
    xi = np.asarray(items_feature, np.float32)
    xb = np.asarray(bundles_feature, np.float32)

    g_il, g_bl, g_bi = _prep(dict(
        il_rows=il_rows, il_cols=il_cols, il_vals=il_vals,
        bl_rows=bl_rows, bl_cols=bl_cols, bl_vals=bl_vals,
        bi_rows=bi_rows, bi_cols=bi_cols, bi_vals=bi_vals))

    if _compiled is None:
        _compiled = _build_program(g_il, g_bl, g_bi)
    nc = _compiled

    # host tables (transposed, dealt order, padded)
    def dealt_T(x, per, pad):
        n = x.shape[0]
        out = np.zeros((64, NC * pad, 1), np.float32)
        g = np.arange(n)
        out[:, (g % NC) * pad + g // NC, 0] = x.T
        return out

    xA_T = dealt_T(xu, A_PER, A_PAD)
    xBi_T = dealt_T(xi, IB_PER, IB_PAD)
    xBb_T = dealt_T(xb, BB_PER, BB_PAD)

    iota_np = np.tile(np.arange(128, dtype=np.float32),
                      (128, 1)).astype(BF16)
    ident_np = np.eye(128, dtype=np.float32)

    def x0_slices(xa, xbs, padA, padB):
        out = np.zeros((NC, padA + padB, D), np.float32)
        ga = np.arange(xa.shape[0])
        out[ga % NC, ga // NC] = xa
        gb = np.arange(xbs.shape[0])
        out[gb % NC, padA + gb // NC] = xbs
        return out

    x0_il = x0_slices(xu, xi, A_PAD, IB_PAD)
    x0_bl = x0_slices(xu, xb, A_PAD, BB_PAD)

    in_maps = []
    for c in range(NC):
        m = {"xA_T": xA_T, "xBi_T": xBi_T, "xBb_T": xBb_T,
             "x0_il": x0_il[c], "x0_bl": x0_bl[c],
             "iota_d": iota_np, "ident_d": ident_np}
        for g in (g_il, g_bl, g_bi):
            m[f"{g.name}_idx"] = g.idx16[c]
            m[f"{g.name}_rows"] = g.rows_f[c]
            m[f"{g.name}_vals"] = g.vals_f[c]
        in_maps.append(m)

    res = run_bass_kernel_spmd(nc, in_maps, core_ids=list(range(NC)))
    kernel.last_exec_ns = res.exec_time_ns

    il_acc = np.stack([res.results[c]["il_acc_out"] for c in range(NC)])
    bl_acc = np.stack([res.results[c]["bl_acc_out"] for c in range(NC)])
    bi_o = np.stack([res.results[c]["bi_out"] for c in range(NC)])

    gu = np.arange(U)
    gi = np.arange(I)
    gb = np.arange(B)
    il_users = il_acc[gu % NC, gu // NC]
    bl_users = bl_acc[gu % NC, gu // NC]
    il_bundles = bi_o[gb % NC, gb // NC]
    bl_bundles = bl_acc[gb % NC, A_PAD + gb // NC]
    return np.concatenate([il_users, bl_users, il_bundles, bl_bundles], 0)


# revision 19
# speedup vs baseline: 1.2313x; 1.2313x over previous
"""Trainium2 Bass kernel for 2-layer bipartite GNN propagation (MDCLBR).

Design (v3):
- Dest rows of each graph are dealt round-robin across the 8 cores
  (side-blocked: A=users then B=items/bundles, each side padded to a tile
  boundary), so every core sees a statistically identical workload and the
  SPMD max-over-cores chunk padding is small.
- Source feature tables are stored TRANSPOSED ([feat, node] fp32) and kept
  resident in SBUF, split in two bucket-halves on partition halves 0-63 /
  64-127.  Edges are gathered with the gpsimd compute gather (ap_gather):
  partition f receives feature f of each edge's source node.  The two
  partition halves gather two independent chunk streams (one per bucket).
- Per 128-edge chunk: PE transposes the gathered [64,128] block into PSUM,
  ACT copies it to SBUF as bf16, DVE builds a scaled one-hot selection
  matrix (iota is_equal row * val, bf16), and the PE accumulates the
  segment sum in PSUM via matmul.
- Per dest tile: ACT scales/copies PSUM, computes the squared-row-norm with
  an accumulating Square and Rsqrt, DVE updates the layer accumulator.
  Layer-1 features (and the item accumulator for the BI aggregation) are
  written back transposed; AllGather assembles the next layer's tables.
"""
import sys
sys.path.insert(0, '/opt/trn_rl_repo')
import numpy as np
import ml_dtypes

U, I, B, D = 50000, 40000, 20000, 64
NC = 8
BF16 = ml_dtypes.bfloat16


def _pad_tiles(per):
    return -(-per // 128) * 128


# side-block geometry (slots per core)
A_PER, A_PAD = U // NC, _pad_tiles(U // NC)          # 6250, 6272
IB_PER, IB_PAD = I // NC, _pad_tiles(I // NC)        # 5000, 5120
BB_PER, BB_PAD = B // NC, _pad_tiles(B // NC)        # 2500, 2560
IL_ROWS = A_PAD + IB_PAD                             # 11392
BL_ROWS = A_PAD + BB_PAD                             # 8832
BI_ROWS = BB_PAD                                     # 2560

_compiled = None


def _deal(vec, per, pad):
    """global side-row -> (core, slot)"""
    return vec % NC, vec // NC


def _perm_order(n, per, pad):
    """host-side: permuted table row p = core*pad + slot -> global row, and
    inverse map global -> table row"""
    g = np.arange(n)
    tab = (g % NC) * pad + g // NC
    return tab


class _Phase:
    """One (spmm, dest-side) phase: tiles [t0, t1) of the spmm's local tile
    space, gathering from a 2-half bucket table."""
    __slots__ = ('t0', 't1', 'half_span', 'supers', 'C', 'name', 'base_chunk')


def _layout_phase(core, slot, trow, vals, t0, t1, half_span, sup_tiles, name):
    """Chunk layout for one phase.

    core/slot: dest (core, slot) per edge; trow: source table row (dealt
    order) per edge; half h = trow >= half_span.
    Returns phase descriptor + per-edge placement arrays:
      chunk id (global within phase, 0..C), partition, idx value.
    """
    t = slot // 128
    h = (trow >= half_span).astype(np.int64)
    idxv = trow - h * half_span
    T = t1 - t0
    key = ((core * T + (t - t0)) * 2 + h)
    counts = np.bincount(key, minlength=NC * T * 2).reshape(NC, T, 2)
    K = -(-counts.max(axis=0) // 128)              # [T, 2]
    # supers: group tiles; per super, two streams (h0, h1) padded to equal
    # chunk count
    ph = _Phase()
    ph.t0, ph.t1, ph.half_span, ph.name = t0, t1, half_span, name
    supers = []
    choff = 0           # global chunk counter (phase-local)
    chunk_of_block = {}
    for s0 in range(0, T, sup_tiles):
        ts = list(range(s0, min(s0 + sup_tiles, T)))
        k0 = int(K[ts, 0].sum())
        k1 = int(K[ts, 1].sum())
        nk = max(k0, k1, 1)
        sup = {'nk': nk, 'tiles': [], 'choff': choff}
        # stream positions: h stream chunk j -> gather col j
        pos = [0, 0]
        for tt in ts:
            tb = []
            for h_ in (0, 1):
                kk = int(K[tt, h_])
                if kk:
                    tb.append((h_, kk, pos[h_]))
                    chunk_of_block[(tt, h_)] = (choff, pos[h_])
                    pos[h_] += kk
            sup['tiles'].append((tt + t0, tb))
        supers.append(sup)
        choff += nk
    ph.supers = supers
    ph.C = choff
    # per-edge placement
    order = np.argsort(key, kind='stable')
    skey = key[order]
    gstart = np.zeros(NC * T * 2, np.int64)
    np.cumsum(counts.reshape(-1)[:-1], out=gstart[1:])
    within = np.arange(len(order)) - gstart[skey]
    so_t = (t - t0)[order]
    so_h = h[order]
    # chunk position within the super's stream
    bo = np.array([chunk_of_block.get((tt, hh), (0, 0))
                   for tt, hh in zip(so_t, so_h)])
    sup_choff = bo[:, 0] if len(bo) else np.zeros(0, np.int64)
    stream_pos = bo[:, 1] if len(bo) else np.zeros(0, np.int64)
    chunk = sup_choff + stream_pos + within // 128
    part = within % 128
    return ph, order, so_h, chunk, part, idxv[order], vals[order], \
        (slot % 128)[order]


class _Graph:
    __slots__ = ('name', 'nc_rows', 'phases', 'idx16', 'rows_f', 'vals_f',
                 'Cidx', 'C')


def _build_graph(name, rows_core, rows_slot, trow, vals, nc_rows, phase_specs):
    """phase_specs: list of (t0, t1, half_span, sup_tiles, edge_mask, pname)"""
    g = _Graph()
    g.name, g.nc_rows = name, nc_rows
    g.phases = []
    placements = []
    C = 0
    for (t0, t1, half_span, sup_tiles, mask, pname) in phase_specs:
        ph, order, so_h, chunk, part, idxv, v, r128 = _layout_phase(
            rows_core[mask], rows_slot[mask], trow[mask], vals[mask],
            t0, t1, half_span, sup_tiles, pname)
        ph.base_chunk = C
        placements.append((rows_core[mask][order], so_h, chunk + C, part,
                           idxv, v, r128))
        # shift super choffs to global chunk ids
        for s in ph.supers:
            s['choff'] += C
        C += ph.C
        g.phases.append(ph)
    g.C = C
    # idx table: [NC, 128, C*8] int16 (per 16-part group wrapped, stream h in
    # partition halves, replicated x4 within half)
    idx16 = np.zeros((NC, 128, C * 8), np.int16)
    rows_f = np.zeros((NC, 128, 2 * C), np.float32)
    vals_f = np.zeros((NC, 128, 2 * C), np.float32)
    for (ecore, eh, echunk, epart, eidx, ev, er) in placements:
        rows_f[ecore, epart, 2 * echunk + eh] = er.astype(np.float32)
        vals_f[ecore, epart, 2 * echunk + eh] = ev.astype(np.float32)
        col = echunk * 8 + epart // 16
        prow = (epart % 16) + (eh * 64)
        for rep in range(4):
            idx16[ecore, rep * 16 + prow, col] = eidx.astype(np.int16)
    g.idx16, g.rows_f, g.vals_f = idx16, rows_f, vals_f
    return g


def _prep(inputs):
    il_rows = np.asarray(inputs['il_rows']).astype(np.int64)
    il_cols = np.asarray(inputs['il_cols']).astype(np.int64)
    il_vals = np.asarray(inputs['il_vals']).astype(np.float32)
    bl_rows = np.asarray(inputs['bl_rows']).astype(np.int64)
    bl_cols = np.asarray(inputs['bl_cols']).astype(np.int64)
    bl_vals = np.asarray(inputs['bl_vals']).astype(np.float32)
    bi_rows = np.asarray(inputs['bi_rows']).astype(np.int64)
    bi_cols = np.asarray(inputs['bi_cols']).astype(np.int64)
    bi_vals = np.asarray(inputs['bi_vals']).astype(np.float32)

    def dest_map(r, nA, padA, padB):
        isB = r >= nA
        q = np.where(isB, r - nA, r)
        return q % NC, np.where(isB, padA + q // NC, q // NC)

    def src_map(c, nA, padA_t, padB_t):
        """source table row in side-dealt order; A table rows [0, NC*padA_t),
        B table rows offset 0 in their own table."""
        isB = c >= nA
        q = np.where(isB, c - nA, c)
        return isB, (q % NC) * np.where(isB, padB_t, padA_t) + q // NC

    # ---- il ----
    core, slot = dest_map(il_rows, U, A_PAD, IB_PAD)
    isB, trow = src_map(il_cols, U, A_PAD, IB_PAD)
    TA, TB = A_PAD // 128, IB_PAD // 128     # 49, 40
    # phase alpha: dest-B tiles (cols are A-side, table NTA = NC*A_PAD=50176)
    # phase beta: dest-A tiles (cols B-side, table NC*IB_PAD = 40960)
    NTA, NTB_il = NC * A_PAD, NC * IB_PAD
    g_il = _build_graph(
        'il', core, slot, np.where(isB, trow, trow), il_vals, IL_ROWS,
        [(TA, TA + TB, NTA // 2, 2, ~isB, 'a'),      # dest-B <- src A
         (0, TA, NTB_il // 2, 3, isB, 'b')])         # dest-A <- src B

    # ---- bl ----
    core, slot = dest_map(bl_rows, U, A_PAD, BB_PAD)
    isB, trow = src_map(bl_cols, U, A_PAD, BB_PAD)
    TBB = BB_PAD // 128                      # 20
    NTB_bl = NC * BB_PAD                     # 20480
    g_bl = _build_graph(
        'bl', core, slot, trow, bl_vals, BL_ROWS,
        [(TA, TA + TBB, NTA // 2, 2, ~isB, 'a'),
         (0, TA, NTB_bl // 2, 3, isB, 'b')])

    # ---- bi ---- dest bundles (side-A of its own space), src = il item acc
    core, slot = bi_rows % NC, bi_rows // NC
    trow = (bi_cols % NC) * IB_PAD + bi_cols // NC
    g_bi = _build_graph(
        'bi', core, slot, trow, bi_vals, BI_ROWS,
        [(0, TBB, NTB_il // 2, 1, np.ones(len(bi_rows), bool), 'a')])

    return g_il, g_bl, g_bi


def _build_program(g_il, g_bl, g_bi):
    from concourse import mybir, bacc
    import concourse.tile as tile

    f32, bf16, i16 = mybir.dt.float32, mybir.dt.bfloat16, mybir.dt.int16
    ACT = mybir.ActivationFunctionType
    nc = bacc.Bacc("TRN2", target_bir_lowering=False, debug=False,
                   num_devices=NC)

    NTA = NC * A_PAD          # 50176 user table rows
    NTB_il = NC * IB_PAD      # 40960
    NTB_bl = NC * BB_PAD      # 20480

    # host-provided transposed fp32 tables + misc
    xA_T = nc.dram_tensor("xA_T", [64, NTA, 1], f32, kind="ExternalInput")
    xBi_T = nc.dram_tensor("xBi_T", [64, NTB_il, 1], f32, kind="ExternalInput")
    xBb_T = nc.dram_tensor("xBb_T", [64, NTB_bl, 1], f32, kind="ExternalInput")
    x0_il = nc.dram_tensor("x0_il", [IL_ROWS, D], f32, kind="ExternalInput")
    x0_bl = nc.dram_tensor("x0_bl", [BL_ROWS, D], f32, kind="ExternalInput")
    iota_d = nc.dram_tensor("iota_d", [128, 128], bf16, kind="ExternalInput")
    ident_d = nc.dram_tensor("ident_d", [128, 128], f32, kind="ExternalInput")
    ins_t = {}
    for g in (g_il, g_bl, g_bi):
        ins_t[g.name] = (
            nc.dram_tensor(f"{g.name}_idx", [128, g.C * 8], i16,
                           kind="ExternalInput"),
            nc.dram_tensor(f"{g.name}_rows", [128, 2 * g.C], f32,
                           kind="ExternalInput"),
            nc.dram_tensor(f"{g.name}_vals", [128, 2 * g.C], f32,
                           kind="ExternalInput"),
        )
    il_acc_mid = nc.dram_tensor("il_acc_mid", [IL_ROWS, D], f32)
    bl_acc_mid = nc.dram_tensor("bl_acc_mid", [BL_ROWS, D], f32)
    il_acc_out = nc.dram_tensor("il_acc_out", [IL_ROWS, D], f32,
                                kind="ExternalOutput")
    bl_acc_out = nc.dram_tensor("bl_acc_out", [BL_ROWS, D], f32,
                                kind="ExternalOutput")
    bi_out = nc.dram_tensor("bi_out", [BI_ROWS, D], f32,
                            kind="ExternalOutput")

    # AG staging: transposed f1 slices and item-acc slices
    il_f1T = nc.dram_tensor("il_f1T", [64, IL_ROWS], f32)
    il_f1T_full = nc.dram_tensor("il_f1T_full", [64 * NC, IL_ROWS], f32,
                                 addr_space="Shared")
    bl_f1T = nc.dram_tensor("bl_f1T", [64, BL_ROWS], f32)
    bl_f1T_full = nc.dram_tensor("bl_f1T_full", [64 * NC, BL_ROWS], f32,
                                 addr_space="Shared")
    gbounce = nc.dram_tensor("gbounce", [64, 4 * 8192], f32)
    accT = nc.dram_tensor("accT", [64, IB_PAD], f32)
    accT_full = nc.dram_tensor("accT_full", [64 * NC, IB_PAD], f32,
                               addr_space="Shared")
    RG = [list(range(NC))]

    with tile.TileContext(nc) as tc:
        with (
            tc.tile_pool(name="const", bufs=1) as cpool,
            tc.tile_pool(name="tabs", bufs=1) as tabpool,
            tc.tile_pool(name="meta", bufs=2) as mpool,
            tc.tile_pool(name="idx", bufs=3) as ipool,
            tc.tile_pool(name="gath", bufs=2) as gpool,
            tc.tile_pool(name="gdn", bufs=2) as dpool,
            tc.tile_pool(name="sel", bufs=6) as spool,
            tc.tile_pool(name="gcast", bufs=6) as bpool,
            tc.tile_pool(name="ptr", bufs=4, space="PSUM") as trpool,
            tc.tile_pool(name="ptrf", bufs=2, space="PSUM") as trfpool,
            tc.tile_pool(name="pacc", bufs=2, space="PSUM") as ppool,
            tc.tile_pool(name="feats", bufs=3) as fpool,
            tc.tile_pool(name="nrm", bufs=4) as npool,
        ):
            iota_b = cpool.tile([128, 128], bf16)
            nc.sync.dma_start(iota_b[:], iota_d[:])
            ident = cpool.tile([128, 128], f32)
            nc.sync.dma_start(ident[:], ident_d[:])

            def load_table_host(src, span2):
                """load a host transposed table into [128, span, 1] tile"""
                t = tabpool.tile([128, span2, 1], f32, tag="tab")
                nc.sync.dma_start(t[0:64, :, :], src[:, 0:span2, :])
                nc.sync.dma_start(t[64:128, :, :], src[:, span2:2 * span2, :])
                return t

            def load_table_ag(agfull, nc_rows, c0, c1, span2):
                """assemble table from AG output [64*NC, nc_rows], cols
                [c0, c1) of each 64-row band -> [128, span2, 1] (2 halves)"""
                t = tabpool.tile([128, span2, 1], f32, tag="tab")
                w = c1 - c0
                for c in range(NC):
                    lo, hi = c * w, (c + 1) * w
                    for hb in (0, 1):
                        s0, s1 = hb * span2, (hb + 1) * span2
                        # intersect [lo,hi) with [s0,s1)
                        a, b = max(lo, s0), min(hi, s1)
                        if a < b:
                            nc.sync.dma_start(
                                t[hb * 64:hb * 64 + 64, a - s0:b - s0, 0],
                                agfull[c * 64:(c + 1) * 64,
                                       c0 + (a - lo):c0 + (b - lo)])
                return t

            def spmm_phase(g, ph, tab, scale, layer, acc_mid, x0_dram,
                           f1T_dram, accT_dram, acc_out):
                idx_d, rows_d, vals_d = ins_t[g.name]
                cb = 2 * ph.base_chunk
                rows_sb = mpool.tile([128, 2 * ph.C], f32, tag="rows")
                vals_sb = mpool.tile([128, 2 * ph.C], f32, tag="vals")
                nc.sync.dma_start(rows_sb[:], rows_d[:, cb:cb + 2 * ph.C])
                nc.sync.dma_start(vals_sb[:], vals_d[:, cb:cb + 2 * ph.C])
                span2 = ph.half_span
                for si, sup in enumerate(ph.supers):
                    nk = sup['nk']
                    gbo = (si % 4) * 8192
                    ioff = sup['choff']
                    idx_t = ipool.tile([128, nk * 8], i16, tag="idx")
                    nc.sync.dma_start(idx_t[:],
                                      idx_d[:, ioff * 8:(ioff + nk) * 8])
                    g_t = gpool.tile([128, nk * 128, 1], f32, tag="g")
                    nc.gpsimd.ap_gather(
                        out_ap=g_t[:], in_ap=tab[:],
                        idxs_ap=idx_t[:], channels=128, num_elems=span2,
                        d=1, num_idxs=nk * 128)
                    nk1 = sum(kk for _, tb_ in sup['tiles']
                              for hh, kk, _ in tb_ if hh == 1)
                    g_d = None
                    if nk1 > 0:
                        g_d = dpool.tile([64, nk * 128], f32, tag="gd")
                        nc.sync.dma_start(gbounce[:, gbo:gbo + nk1 * 128],
                                          g_t[64:128, 0:nk1 * 128, 0])
                        nc.sync.dma_start(g_d[:, 0:nk1 * 128],
                                          gbounce[:, gbo:gbo + nk1 * 128])
                    for tt, tb in sup['tiles']:
                        nchunks = sum(kk for _, kk, _ in tb)
                        if nchunks == 0:
                            continue
                        psum_t = ppool.tile([128, D], f32, tag="ps")
                        done = 0
                        for h_, kk, pos0 in tb:
                            base = sup['choff'] + pos0
                            for k in range(kk):
                                cid = 2 * (base + k) + h_
                                co = (pos0 + k) * 128
                                ptr = trpool.tile([128, 64], f32, tag="tr")
                                src_ap = (g_t[0:64, co:co + 128, 0]
                                          if h_ == 0 else
                                          g_d[:, co:co + 128])
                                nc.tensor.matmul(
                                    ptr[:], src_ap, ident[0:64, 0:64],
                                    is_transpose=True,
                                    skip_group_check=True)
                                g_b = bpool.tile([128, 64], bf16, tag="gb")
                                nc.scalar.activation(g_b[:], ptr[:], ACT.Copy)
                                s_t = spool.tile([128, 128], bf16, tag="s")
                                nc.vector.tensor_scalar(
                                    out=s_t[:], in0=iota_b[:],
                                    scalar1=rows_sb[:, cid - cb:cid - cb + 1],
                                    scalar2=vals_sb[:, cid - cb:cid - cb + 1],
                                    op0=mybir.AluOpType.is_equal,
                                    op1=mybir.AluOpType.mult)
                                nc.tensor.matmul(
                                    psum_t[:], s_t[:], g_b[:],
                                    start=(done == 0),
                                    stop=(done == nchunks - 1),
                                    skip_group_check=True)
                                done += 1
                        # ---- tile epilogue ----
                        if layer is None:
                            o_t = fpool.tile([128, D], f32, tag="f")
                            nc.scalar.activation(o_t[:], psum_t[:], ACT.Copy)
                            nc.sync.dma_start(
                                bi_out[tt * 128:(tt + 1) * 128, :], o_t[:])
                            continue
                        f_s = fpool.tile([128, D], f32, tag="f")
                        nc.scalar.activation(f_s[:], psum_t[:], ACT.Copy,
                                             scale=scale)
                        sq = npool.tile([128, D], f32, tag="sq")
                        n2 = npool.tile([128, 1], f32, tag="n2")
                        nc.scalar.activation(sq[:], f_s[:], ACT.Square,
                                             accum_out=n2[:])
                        nr = npool.tile([128, 1], f32, tag="nr")
                        nc.scalar.activation(nr[:], n2[:], ACT.Sqrt)
                        nc.vector.tensor_scalar_max(nr[:], nr[:], 1e-12)
                        ri = npool.tile([128, 1], f32, tag="ri")
                        nc.vector.reciprocal(ri[:], nr[:])
                        ao = fpool.tile([128, D], f32, tag="ao")
                        if layer == 0:
                            x0_t = fpool.tile([128, D], f32, tag="x0")
                            nc.sync.dma_start(
                                x0_t[:], x0_dram[tt * 128:(tt + 1) * 128, :])
                            nc.vector.scalar_tensor_tensor(
                                out=ao[:], in0=f_s[:], scalar=ri[:, 0:1],
                                in1=x0_t[:], op0=mybir.AluOpType.mult,
                                op1=mybir.AluOpType.add)
                            nc.sync.dma_start(
                                acc_mid[tt * 128:(tt + 1) * 128, :], ao[:])
                            # write f1 transposed for next layer's table
                            ptr2 = trfpool.tile([64, 128], f32, tag="trf")
                            nc.tensor.matmul(ptr2[:], f_s[:], ident[:],
                                             is_transpose=True,
                                             skip_group_check=True)
                            fT = fpool.tile([64, 128], f32, tag="fT")
                            nc.scalar.activation(fT[:], ptr2[:], ACT.Copy)
                            nc.sync.dma_start(
                                f1T_dram[:, tt * 128:(tt + 1) * 128],
                                fT[:])
                        else:
                            am = fpool.tile([128, D], f32, tag="am")
                            nc.sync.dma_start(
                                am[:], acc_mid[tt * 128:(tt + 1) * 128, :])
                            nc.vector.scalar_tensor_tensor(
                                out=ao[:], in0=f_s[:], scalar=ri[:, 0:1],
                                in1=am[:], op0=mybir.AluOpType.mult,
                                op1=mybir.AluOpType.add)
                            nc.sync.dma_start(
                                acc_out[tt * 128:(tt + 1) * 128, :],
                                ao[:])
                            if accT_dram is not None and tt >= g.phases[0].t0:
                                # item tiles: transposed acc for BI table
                                ptr2 = trfpool.tile([64, 128], f32, tag="trf")
                                nc.tensor.matmul(ptr2[:], ao[:], ident[:],
                                                 is_transpose=True,
                                                 skip_group_check=True)
                                fT = fpool.tile([64, 128], f32, tag="fT")
                                nc.scalar.activation(fT[:], ptr2[:], ACT.Copy)
                                t0 = g.phases[0].t0
                                nc.sync.dma_start(
                                    accT_dram[:, (tt - t0) * 128:
                                              (tt - t0 + 1) * 128],
                                    fT[:])

            TA = A_PAD // 128

            # P1: users table; il-L1-alpha (item tiles)
            tabA = load_table_host(xA_T, NTA // 2)
            spmm_phase(g_il, g_il.phases[0], tabA, 0.5, 0, il_acc_mid, x0_il,
                       il_f1T, None, il_acc_out)
            # P2: bl-L1-alpha (bundle tiles) -- same users table
            spmm_phase(g_bl, g_bl.phases[0], tabA, 0.5, 0, bl_acc_mid, x0_bl,
                       bl_f1T, None, bl_acc_out)
            # P3: items-x table; il-L1-beta (user tiles)
            tabBi = load_table_host(xBi_T, NTB_il // 2)
            spmm_phase(g_il, g_il.phases[1], tabBi, 0.5, 0, il_acc_mid, x0_il,
                       il_f1T, None, il_acc_out)
            nc.gpsimd.collective_compute(
                "AllGather", mybir.AluOpType.bypass, ins=[il_f1T[:]],
                outs=[il_f1T_full[:]], replica_groups=RG)
            # P4: bundles-x table; bl-L1-beta (user tiles)
            tabBb = load_table_host(xBb_T, NTB_bl // 2)
            spmm_phase(g_bl, g_bl.phases[1], tabBb, 0.5, 0, bl_acc_mid, x0_bl,
                       bl_f1T, None, bl_acc_out)
            nc.gpsimd.collective_compute(
                "AllGather", mybir.AluOpType.bypass, ins=[bl_f1T[:]],
                outs=[bl_f1T_full[:]], replica_groups=RG)
            # P5: il-f1 user table; il-L2-alpha (item tiles)
            tabf1A = load_table_ag(il_f1T_full, IL_ROWS, 0, A_PAD, NTA // 2)
            spmm_phase(g_il, g_il.phases[0], tabf1A, 1.0 / 3, 1, il_acc_mid,
                       None, None, accT, il_acc_out)
            nc.gpsimd.collective_compute(
                "AllGather", mybir.AluOpType.bypass, ins=[accT[:]],
                outs=[accT_full[:]], replica_groups=RG)
            # P6: il-f1 item table; il-L2-beta (user tiles)
            tabf1B = load_table_ag(il_f1T_full, IL_ROWS, A_PAD, IL_ROWS,
                                   NTB_il // 2)
            spmm_phase(g_il, g_il.phases[1], tabf1B, 1.0 / 3, 1, il_acc_mid,
                       None, None, None, il_acc_out)
            # P7: bl-f1 user table; bl-L2-alpha
            tabg1A = load_table_ag(bl_f1T_full, BL_ROWS, 0, A_PAD, NTA // 2)
            spmm_phase(g_bl, g_bl.phases[0], tabg1A, 1.0 / 3, 1, bl_acc_mid,
                       None, None, None, bl_acc_out)
            # P8: bl-f1 bundle table; bl-L2-beta
            tabg1B = load_table_ag(bl_f1T_full, BL_ROWS, A_PAD, BL_ROWS,
                                   NTB_bl // 2)
            spmm_phase(g_bl, g_bl.phases[1], tabg1B, 1.0 / 3, 1, bl_acc_mid,
                       None, None, None, bl_acc_out)
            # P9: bi aggregation from item acc
            tabacc = load_table_ag(accT_full, IB_PAD, 0, IB_PAD, NTB_il // 2)
            spmm_phase(g_bi, g_bi.phases[0], tabacc, 1.0, None, None, None,
                       None, None, None)

    nc.compile()
    return nc


def kernel(users_feature, items_feature, bundles_feature,
           il_rows, il_cols, il_vals,
           bl_rows, bl_cols, bl_vals,
           bi_rows, bi_cols, bi_vals):
    global _compiled
    from concourse.bass_utils import run_bass_kernel_spmd

    xu = np.asarray(users_feature, np.float32)
    xi = np.asarray(items_feature, np.float32)
    xb = np.asarray(bundles_feature, np.float32)

    g_il, g_bl, g_bi = _prep(dict(
        il_rows=il_rows, il_cols=il_cols, il_vals=il_vals,
        bl_rows=bl_rows, bl_cols=bl_cols, bl_vals=bl_vals,
        bi_rows=bi_rows, bi_cols=bi_cols, bi_vals=bi_vals))

    if _compiled is None:
        _compiled = _build_program(g_il, g_bl, g_bi)
    nc = _compiled

    # host tables (transposed, dealt order, padded)
    def dealt_T(x, per, pad):
        n = x.shape[0]
        out = np.zeros((64, NC * pad, 1), np.float32)
        g = np.arange(n)
        out[:, (g % NC) * pad + g // NC, 0] = x.T
        return out

    xA_T = dealt_T(xu, A_PER, A_PAD)
    xBi_T = dealt_T(xi, IB_PER, IB_PAD)
    xBb_T = dealt_T(xb, BB_PER, BB_PAD)

    iota_np = np.tile(np.arange(128, dtype=np.float32),
                      (128, 1)).astype(BF16)
    ident_np = np.eye(128, dtype=np.float32)

    def x0_slices(xa, xbs, padA, padB):
        out = np.zeros((NC, padA + padB, D), np.float32)
        ga = np.arange(xa.shape[0])
        out[ga % NC, ga // NC] = xa
        gb = np.arange(xbs.shape[0])
        out[gb % NC, padA + gb // NC] = xbs
        return out

    x0_il = x0_slices(xu, xi, A_PAD, IB_PAD)
    x0_bl = x0_slices(xu, xb, A_PAD, BB_PAD)

    in_maps = []
    for c in range(NC):
        m = {"xA_T": xA_T, "xBi_T": xBi_T, "xBb_T": xBb_T,
             "x0_il": x0_il[c], "x0_bl": x0_bl[c],
             "iota_d": iota_np, "ident_d": ident_np}
        for g in (g_il, g_bl, g_bi):
            m[f"{g.name}_idx"] = g.idx16[c]
            m[f"{g.name}_rows"] = g.rows_f[c]
            m[f"{g.name}_vals"] = g.vals_f[c]
        in_maps.append(m)

    res = run_bass_kernel_spmd(nc, in_maps, core_ids=list(range(NC)))
    kernel.last_exec_ns = res.exec_time_ns

    il_acc = np.stack([res.results[c]["il_acc_out"] for c in range(NC)])
    bl_acc = np.stack([res.results[c]["bl_acc_out"] for c in range(NC)])
    bi_o = np.stack([res.results[c]["bi_out"] for c in range(NC)])

    gu = np.arange(U)
    gi = np.arange(I)
    gb = np.arange(B)
    il_users = il_acc[gu % NC, gu // NC]
    bl_users = bl_acc[gu % NC, gu // NC]
    il_bundles = bi_o[gb % NC, gb // NC]
    bl_bundles = bl_acc[gb % NC, A_PAD + gb // NC]
    return np.concatenate([il_users, bl_users, il_bundles, bl_bundles], 0)


# revision 20
# speedup vs baseline: 1.2326x; 1.0011x over previous
"""Trainium2 Bass kernel for 2-layer bipartite GNN propagation (MDCLBR).

Design (v3):
- Dest rows of each graph are dealt round-robin across the 8 cores
  (side-blocked: A=users then B=items/bundles, each side padded to a tile
  boundary), so every core sees a statistically identical workload and the
  SPMD max-over-cores chunk padding is small.
- Source feature tables are stored TRANSPOSED ([feat, node] fp32) and kept
  resident in SBUF, split in two bucket-halves on partition halves 0-63 /
  64-127.  Edges are gathered with the gpsimd compute gather (ap_gather):
  partition f receives feature f of each edge's source node.  The two
  partition halves gather two independent chunk streams (one per bucket).
- Per 128-edge chunk: PE transposes the gathered [64,128] block into PSUM,
  ACT copies it to SBUF as bf16, DVE builds a scaled one-hot selection
  matrix (iota is_equal row * val, bf16), and the PE accumulates the
  segment sum in PSUM via matmul.
- Per dest tile: ACT scales/copies PSUM, computes the squared-row-norm with
  an accumulating Square and Rsqrt, DVE updates the layer accumulator.
  Layer-1 features (and the item accumulator for the BI aggregation) are
  written back transposed; AllGather assembles the next layer's tables.
"""
import sys
sys.path.insert(0, '/opt/trn_rl_repo')
import numpy as np
import ml_dtypes

U, I, B, D = 50000, 40000, 20000, 64
NC = 8
BF16 = ml_dtypes.bfloat16


def _pad_tiles(per):
    return -(-per // 128) * 128


# side-block geometry (slots per core)
A_PER, A_PAD = U // NC, _pad_tiles(U // NC)          # 6250, 6272
IB_PER, IB_PAD = I // NC, _pad_tiles(I // NC)        # 5000, 5120
BB_PER, BB_PAD = B // NC, _pad_tiles(B // NC)        # 2500, 2560
IL_ROWS = A_PAD + IB_PAD                             # 11392
BL_ROWS = A_PAD + BB_PAD                             # 8832
BI_ROWS = BB_PAD                                     # 2560

_compiled = None


def _deal(vec, per, pad):
    """global side-row -> (core, slot)"""
    return vec % NC, vec // NC


def _perm_order(n, per, pad):
    """host-side: permuted table row p = core*pad + slot -> global row, and
    inverse map global -> table row"""
    g = np.arange(n)
    tab = (g % NC) * pad + g // NC
    return tab


class _Phase:
    """One (spmm, dest-side) phase: tiles [t0, t1) of the spmm's local tile
    space, gathering from a 2-half bucket table."""
    __slots__ = ('t0', 't1', 'half_span', 'supers', 'C', 'name', 'base_chunk')


def _layout_phase(core, slot, trow, vals, t0, t1, half_span, sup_tiles, name):
    """Chunk layout for one phase.

    core/slot: dest (core, slot) per edge; trow: source table row (dealt
    order) per edge; half h = trow >= half_span.
    Returns phase descriptor + per-edge placement arrays:
      chunk id (global within phase, 0..C), partition, idx value.
    """
    t = slot // 128
    h = (trow >= half_span).astype(np.int64)
    idxv = trow - h * half_span
    T = t1 - t0
    key = ((core * T + (t - t0)) * 2 + h)
    counts = np.bincount(key, minlength=NC * T * 2).reshape(NC, T, 2)
    K = -(-counts.max(axis=0) // 128)              # [T, 2]
    # supers: group tiles; per super, two streams (h0, h1) padded to equal
    # chunk count
    ph = _Phase()
    ph.t0, ph.t1, ph.half_span, ph.name = t0, t1, half_span, name
    supers = []
    choff = 0           # global chunk counter (phase-local)
    chunk_of_block = {}
    for s0 in range(0, T, sup_tiles):
        ts = list(range(s0, min(s0 + sup_tiles, T)))
        k0 = int(K[ts, 0].sum())
        k1 = int(K[ts, 1].sum())
        nk = max(k0, k1, 1)
        sup = {'nk': nk, 'tiles': [], 'choff': choff}
        # stream positions: h stream chunk j -> gather col j
        pos = [0, 0]
        for tt in ts:
            tb = []
            for h_ in (0, 1):
                kk = int(K[tt, h_])
                if kk:
                    tb.append((h_, kk, pos[h_]))
                    chunk_of_block[(tt, h_)] = (choff, pos[h_])
                    pos[h_] += kk
            sup['tiles'].append((tt + t0, tb))
        supers.append(sup)
        choff += nk
    ph.supers = supers
    ph.C = choff
    # per-edge placement
    order = np.argsort(key, kind='stable')
    skey = key[order]
    gstart = np.zeros(NC * T * 2, np.int64)
    np.cumsum(counts.reshape(-1)[:-1], out=gstart[1:])
    within = np.arange(len(order)) - gstart[skey]
    so_t = (t - t0)[order]
    so_h = h[order]
    # chunk position within the super's stream
    bo = np.array([chunk_of_block.get((tt, hh), (0, 0))
                   for tt, hh in zip(so_t, so_h)])
    sup_choff = bo[:, 0] if len(bo) else np.zeros(0, np.int64)
    stream_pos = bo[:, 1] if len(bo) else np.zeros(0, np.int64)
    chunk = sup_choff + stream_pos + within // 128
    part = within % 128
    return ph, order, so_h, chunk, part, idxv[order], vals[order], \
        (slot % 128)[order]


class _Graph:
    __slots__ = ('name', 'nc_rows', 'phases', 'idx16', 'rows_f', 'vals_f',
                 'Cidx', 'C')


def _build_graph(name, rows_core, rows_slot, trow, vals, nc_rows, phase_specs):
    """phase_specs: list of (t0, t1, half_span, sup_tiles, edge_mask, pname)"""
    g = _Graph()
    g.name, g.nc_rows = name, nc_rows
    g.phases = []
    placements = []
    C = 0
    for (t0, t1, half_span, sup_tiles, mask, pname) in phase_specs:
        ph, order, so_h, chunk, part, idxv, v, r128 = _layout_phase(
            rows_core[mask], rows_slot[mask], trow[mask], vals[mask],
            t0, t1, half_span, sup_tiles, pname)
        ph.base_chunk = C
        placements.append((rows_core[mask][order], so_h, chunk + C, part,
                           idxv, v, r128))
        # shift super choffs to global chunk ids
        for s in ph.supers:
            s['choff'] += C
        C += ph.C
        g.phases.append(ph)
    g.C = C
    # idx table: [NC, 128, C*8] int16 (per 16-part group wrapped, stream h in
    # partition halves, replicated x4 within half)
    idx16 = np.zeros((NC, 128, C * 8), np.int16)
    rows_f = np.zeros((NC, 128, 2 * C), np.float32)
    vals_f = np.zeros((NC, 128, 2 * C), np.float32)
    for (ecore, eh, echunk, epart, eidx, ev, er) in placements:
        rows_f[ecore, epart, 2 * echunk + eh] = er.astype(np.float32)
        vals_f[ecore, epart, 2 * echunk + eh] = ev.astype(np.float32)
        col = echunk * 8 + epart // 16
        prow = (epart % 16) + (eh * 64)
        for rep in range(4):
            idx16[ecore, rep * 16 + prow, col] = eidx.astype(np.int16)
    g.idx16, g.rows_f, g.vals_f = idx16, rows_f, vals_f
    return g


def _prep(inputs):
    il_rows = np.asarray(inputs['il_rows']).astype(np.int64)
    il_cols = np.asarray(inputs['il_cols']).astype(np.int64)
    il_vals = np.asarray(inputs['il_vals']).astype(np.float32)
    bl_rows = np.asarray(inputs['bl_rows']).astype(np.int64)
    bl_cols = np.asarray(inputs['bl_cols']).astype(np.int64)
    bl_vals = np.asarray(inputs['bl_vals']).astype(np.float32)
    bi_rows = np.asarray(inputs['bi_rows']).astype(np.int64)
    bi_cols = np.asarray(inputs['bi_cols']).astype(np.int64)
    bi_vals = np.asarray(inputs['bi_vals']).astype(np.float32)

    def dest_map(r, nA, padA, padB):
        isB = r >= nA
        q = np.where(isB, r - nA, r)
        return q % NC, np.where(isB, padA + q // NC, q // NC)

    def src_map(c, nA, padA_t, padB_t):
        """source table row in side-dealt order; A table rows [0, NC*padA_t),
        B table rows offset 0 in their own table."""
        isB = c >= nA
        q = np.where(isB, c - nA, c)
        return isB, (q % NC) * np.where(isB, padB_t, padA_t) + q // NC

    # ---- il ----
    core, slot = dest_map(il_rows, U, A_PAD, IB_PAD)
    isB, trow = src_map(il_cols, U, A_PAD, IB_PAD)
    TA, TB = A_PAD // 128, IB_PAD // 128     # 49, 40
    # phase alpha: dest-B tiles (cols are A-side, table NTA = NC*A_PAD=50176)
    # phase beta: dest-A tiles (cols B-side, table NC*IB_PAD = 40960)
    NTA, NTB_il = NC * A_PAD, NC * IB_PAD
    g_il = _build_graph(
        'il', core, slot, np.where(isB, trow, trow), il_vals, IL_ROWS,
        [(TA, TA + TB, NTA // 2, 2, ~isB, 'a'),      # dest-B <- src A
         (0, TA, NTB_il // 2, 3, isB, 'b')])         # dest-A <- src B

    # ---- bl ----
    core, slot = dest_map(bl_rows, U, A_PAD, BB_PAD)
    isB, trow = src_map(bl_cols, U, A_PAD, BB_PAD)
    TBB = BB_PAD // 128                      # 20
    NTB_bl = NC * BB_PAD                     # 20480
    g_bl = _build_graph(
        'bl', core, slot, trow, bl_vals, BL_ROWS,
        [(TA, TA + TBB, NTA // 2, 2, ~isB, 'a'),
         (0, TA, NTB_bl // 2, 3, isB, 'b')])

    # ---- bi ---- dest bundles (side-A of its own space), src = il item acc
    core, slot = bi_rows % NC, bi_rows // NC
    trow = (bi_cols % NC) * IB_PAD + bi_cols // NC
    g_bi = _build_graph(
        'bi', core, slot, trow, bi_vals, BI_ROWS,
        [(0, TBB, NTB_il // 2, 1, np.ones(len(bi_rows), bool), 'a')])

    return g_il, g_bl, g_bi


def _build_program(g_il, g_bl, g_bi):
    from concourse import mybir, bacc
    import concourse.tile as tile

    f32, bf16, i16 = mybir.dt.float32, mybir.dt.bfloat16, mybir.dt.int16
    ACT = mybir.ActivationFunctionType
    nc = bacc.Bacc("TRN2", target_bir_lowering=False, debug=False,
                   num_devices=NC)

    NTA = NC * A_PAD          # 50176 user table rows
    NTB_il = NC * IB_PAD      # 40960
    NTB_bl = NC * BB_PAD      # 20480

    # host-provided transposed fp32 tables + misc
    xA_T = nc.dram_tensor("xA_T", [64, NTA, 1], f32, kind="ExternalInput")
    xBi_T = nc.dram_tensor("xBi_T", [64, NTB_il, 1], f32, kind="ExternalInput")
    xBb_T = nc.dram_tensor("xBb_T", [64, NTB_bl, 1], f32, kind="ExternalInput")
    x0_il = nc.dram_tensor("x0_il", [IL_ROWS, D], f32, kind="ExternalInput")
    x0_bl = nc.dram_tensor("x0_bl", [BL_ROWS, D], f32, kind="ExternalInput")
    iota_d = nc.dram_tensor("iota_d", [128, 128], bf16, kind="ExternalInput")
    ident_d = nc.dram_tensor("ident_d", [128, 128], f32, kind="ExternalInput")
    ins_t = {}
    for g in (g_il, g_bl, g_bi):
        ins_t[g.name] = (
            nc.dram_tensor(f"{g.name}_idx", [128, g.C * 8], i16,
                           kind="ExternalInput"),
            nc.dram_tensor(f"{g.name}_rows", [128, 2 * g.C], f32,
                           kind="ExternalInput"),
            nc.dram_tensor(f"{g.name}_vals", [128, 2 * g.C], f32,
                           kind="ExternalInput"),
        )
    il_acc_mid = nc.dram_tensor("il_acc_mid", [IL_ROWS, D], f32)
    bl_acc_mid = nc.dram_tensor("bl_acc_mid", [BL_ROWS, D], f32)
    il_acc_out = nc.dram_tensor("il_acc_out", [IL_ROWS, D], f32,
                                kind="ExternalOutput")
    bl_acc_out = nc.dram_tensor("bl_acc_out", [BL_ROWS, D], f32,
                                kind="ExternalOutput")
    bi_out = nc.dram_tensor("bi_out", [BI_ROWS, D], f32,
                            kind="ExternalOutput")

    # AG staging: transposed f1 slices and item-acc slices
    il_f1T = nc.dram_tensor("il_f1T", [64, IL_ROWS], f32)
    il_f1T_full = nc.dram_tensor("il_f1T_full", [64 * NC, IL_ROWS], f32,
                                 addr_space="Shared")
    bl_f1T = nc.dram_tensor("bl_f1T", [64, BL_ROWS], f32)
    bl_f1T_full = nc.dram_tensor("bl_f1T_full", [64 * NC, BL_ROWS], f32,
                                 addr_space="Shared")
    gbounce = nc.dram_tensor("gbounce", [64, 4 * 8192], f32)
    accT = nc.dram_tensor("accT", [64, IB_PAD], f32)
    accT_full = nc.dram_tensor("accT_full", [64 * NC, IB_PAD], f32,
                               addr_space="Shared")
    RG = [list(range(NC))]

    with tile.TileContext(nc) as tc:
        with (
            tc.tile_pool(name="const", bufs=1) as cpool,
            tc.tile_pool(name="tabs", bufs=1) as tabpool,
            tc.tile_pool(name="meta", bufs=2) as mpool,
            tc.tile_pool(name="idx", bufs=30) as ipool,
            tc.tile_pool(name="gath", bufs=3) as gpool,
            tc.tile_pool(name="gdn", bufs=2) as dpool,
            tc.tile_pool(name="sel", bufs=6) as spool,
            tc.tile_pool(name="gcast", bufs=6) as bpool,
            tc.tile_pool(name="ptr", bufs=3, space="PSUM") as trpool,
            tc.tile_pool(name="ptrf", bufs=2, space="PSUM") as trfpool,
            tc.tile_pool(name="pacc", bufs=3, space="PSUM") as ppool,
            tc.tile_pool(name="feats", bufs=3) as fpool,
            tc.tile_pool(name="nrm", bufs=4) as npool,
        ):
            iota_b = cpool.tile([128, 128], bf16)
            nc.sync.dma_start(iota_b[:], iota_d[:])
            ident = cpool.tile([128, 128], f32)
            nc.sync.dma_start(ident[:], ident_d[:])

            def load_table_host(src, span2):
                """load a host transposed table into [128, span, 1] tile"""
                t = tabpool.tile([128, span2, 1], f32, tag="tab")
                nc.sync.dma_start(t[0:64, :, :], src[:, 0:span2, :])
                nc.sync.dma_start(t[64:128, :, :], src[:, span2:2 * span2, :])
                return t

            def load_table_ag(agfull, nc_rows, c0, c1, span2):
                """assemble table from AG output [64*NC, nc_rows], cols
                [c0, c1) of each 64-row band -> [128, span2, 1] (2 halves)"""
                t = tabpool.tile([128, span2, 1], f32, tag="tab")
                w = c1 - c0
                for c in range(NC):
                    lo, hi = c * w, (c + 1) * w
                    for hb in (0, 1):
                        s0, s1 = hb * span2, (hb + 1) * span2
                        # intersect [lo,hi) with [s0,s1)
                        a, b = max(lo, s0), min(hi, s1)
                        if a < b:
                            nc.sync.dma_start(
                                t[hb * 64:hb * 64 + 64, a - s0:b - s0, 0],
                                agfull[c * 64:(c + 1) * 64,
                                       c0 + (a - lo):c0 + (b - lo)])
                return t

            def spmm_phase(g, ph, tab, scale, layer, acc_mid, x0_dram,
                           f1T_dram, accT_dram, acc_out):
                idx_d, rows_d, vals_d = ins_t[g.name]
                cb = 2 * ph.base_chunk
                rows_sb = mpool.tile([128, 2 * ph.C], f32, tag="rows")
                vals_sb = mpool.tile([128, 2 * ph.C], f32, tag="vals")
                nc.sync.dma_start(rows_sb[:], rows_d[:, cb:cb + 2 * ph.C])
                nc.sync.dma_start(vals_sb[:], vals_d[:, cb:cb + 2 * ph.C])
                span2 = ph.half_span
                idx_ts = []
                for sup in ph.supers:
                    nk = sup['nk']
                    ioff = sup['choff']
                    it = ipool.tile([128, nk * 8], i16, tag="idx")
                    nc.sync.dma_start(it[:],
                                      idx_d[:, ioff * 8:(ioff + nk) * 8])
                    idx_ts.append(it)
                for si, sup in enumerate(ph.supers):
                    nk = sup['nk']
                    gbo = (si % 4) * 8192
                    ioff = sup['choff']
                    idx_t = idx_ts[si]
                    g_t = gpool.tile([128, nk * 128, 1], f32, tag="g")
                    nc.gpsimd.ap_gather(
                        out_ap=g_t[:], in_ap=tab[:],
                        idxs_ap=idx_t[:], channels=128, num_elems=span2,
                        d=1, num_idxs=nk * 128)
                    nk1 = sum(kk for _, tb_ in sup['tiles']
                              for hh, kk, _ in tb_ if hh == 1)
                    g_d = None
                    if nk1 > 0:
                        g_d = dpool.tile([64, nk * 128], f32, tag="gd")
                        nc.sync.dma_start(gbounce[:, gbo:gbo + nk1 * 128],
                                          g_t[64:128, 0:nk1 * 128, 0])
                        nc.sync.dma_start(g_d[:, 0:nk1 * 128],
                                          gbounce[:, gbo:gbo + nk1 * 128])
                    for tt, tb in sup['tiles']:
                        nchunks = sum(kk for _, kk, _ in tb)
                        if nchunks == 0:
                            continue
                        psum_t = ppool.tile([128, D], f32, tag="ps")
                        done = 0
                        for h_, kk, pos0 in tb:
                            base = sup['choff'] + pos0
                            for k in range(kk):
                                cid = 2 * (base + k) + h_
                                co = (pos0 + k) * 128
                                ptr = trpool.tile([128, 64], f32, tag="tr")
                                src_ap = (g_t[0:64, co:co + 128, 0]
                                          if h_ == 0 else
                                          g_d[:, co:co + 128])
                                nc.tensor.matmul(
                                    ptr[:], src_ap, ident[0:64, 0:64],
                                    is_transpose=True,
                                    skip_group_check=True)
                                g_b = bpool.tile([128, 64], bf16, tag="gb")
                                nc.scalar.activation(g_b[:], ptr[:], ACT.Copy)
                                s_t = spool.tile([128, 128], bf16, tag="s")
                                nc.vector.tensor_scalar(
                                    out=s_t[:], in0=iota_b[:],
                                    scalar1=rows_sb[:, cid - cb:cid - cb + 1],
                                    scalar2=vals_sb[:, cid - cb:cid - cb + 1],
                                    op0=mybir.AluOpType.is_equal,
                                    op1=mybir.AluOpType.mult)
                                nc.tensor.matmul(
                                    psum_t[:], s_t[:], g_b[:],
                                    start=(done == 0),
                                    stop=(done == nchunks - 1),
                                    skip_group_check=True)
                                done += 1
                        # ---- tile epilogue ----
                        if layer is None:
                            o_t = fpool.tile([128, D], f32, tag="f")
                            nc.scalar.activation(o_t[:], psum_t[:], ACT.Copy)
                            nc.sync.dma_start(
                                bi_out[tt * 128:(tt + 1) * 128, :], o_t[:])
                            continue
                        f_s = fpool.tile([128, D], f32, tag="f")
                        nc.scalar.activation(f_s[:], psum_t[:], ACT.Copy,
                                             scale=scale)
                        sq = npool.tile([128, D], f32, tag="sq")
                        n2 = npool.tile([128, 1], f32, tag="n2")
                        nc.scalar.activation(sq[:], f_s[:], ACT.Square,
                                             accum_out=n2[:])
                        nr = npool.tile([128, 1], f32, tag="nr")
                        nc.scalar.activation(nr[:], n2[:], ACT.Sqrt)
                        nc.vector.tensor_scalar_max(nr[:], nr[:], 1e-12)
                        ri = npool.tile([128, 1], f32, tag="ri")
                        nc.vector.reciprocal(ri[:], nr[:])
                        ao = fpool.tile([128, D], f32, tag="ao")
                        if layer == 0:
                            x0_t = fpool.tile([128, D], f32, tag="x0")
                            nc.sync.dma_start(
                                x0_t[:], x0_dram[tt * 128:(tt + 1) * 128, :])
                            nc.vector.scalar_tensor_tensor(
                                out=ao[:], in0=f_s[:], scalar=ri[:, 0:1],
                                in1=x0_t[:], op0=mybir.AluOpType.mult,
                                op1=mybir.AluOpType.add)
                            nc.sync.dma_start(
                                acc_mid[tt * 128:(tt + 1) * 128, :], ao[:])
                            # write f1 transposed for next layer's table
                            ptr2 = trfpool.tile([64, 128], f32, tag="trf")
                            nc.tensor.matmul(ptr2[:], f_s[:], ident[:],
                                             is_transpose=True,
                                             skip_group_check=True)
                            fT = fpool.tile([64, 128], f32, tag="fT")
                            nc.scalar.activation(fT[:], ptr2[:], ACT.Copy)
                            nc.sync.dma_start(
                                f1T_dram[:, tt * 128:(tt + 1) * 128],
                                fT[:])
                        else:
                            am = fpool.tile([128, D], f32, tag="am")
                            nc.sync.dma_start(
                                am[:], acc_mid[tt * 128:(tt + 1) * 128, :])
                            nc.vector.scalar_tensor_tensor(
                                out=ao[:], in0=f_s[:], scalar=ri[:, 0:1],
                                in1=am[:], op0=mybir.AluOpType.mult,
                                op1=mybir.AluOpType.add)
                            nc.sync.dma_start(
                                acc_out[tt * 128:(tt + 1) * 128, :],
                                ao[:])
                            if accT_dram is not None and tt >= g.phases[0].t0:
                                # item tiles: transposed acc for BI table
                                ptr2 = trfpool.tile([64, 128], f32, tag="trf")
                                nc.tensor.matmul(ptr2[:], ao[:], ident[:],
                                                 is_transpose=True,
                                                 skip_group_check=True)
                                fT = fpool.tile([64, 128], f32, tag="fT")
                                nc.scalar.activation(fT[:], ptr2[:], ACT.Copy)
                                t0 = g.phases[0].t0
                                nc.sync.dma_start(
                                    accT_dram[:, (tt - t0) * 128:
                                              (tt - t0 + 1) * 128],
                                    fT[:])

            TA = A_PAD // 128

            # P1: users table; il-L1-alpha (item tiles)
            tabA = load_table_host(xA_T, NTA // 2)
            spmm_phase(g_il, g_il.phases[0], tabA, 0.5, 0, il_acc_mid, x0_il,
                       il_f1T, None, il_acc_out)
            # P2: bl-L1-alpha (bundle tiles) -- same users table
            spmm_phase(g_bl, g_bl.phases[0], tabA, 0.5, 0, bl_acc_mid, x0_bl,
                       bl_f1T, None, bl_acc_out)
            # P3: items-x table; il-L1-beta (user tiles)
            tabBi = load_table_host(xBi_T, NTB_il // 2)
            spmm_phase(g_il, g_il.phases[1], tabBi, 0.5, 0, il_acc_mid, x0_il,
                       il_f1T, None, il_acc_out)
            nc.gpsimd.collective_compute(
                "AllGather", mybir.AluOpType.bypass, ins=[il_f1T[:]],
                outs=[il_f1T_full[:]], replica_groups=RG)
            # P4: bundles-x table; bl-L1-beta (user tiles)
            tabBb = load_table_host(xBb_T, NTB_bl // 2)
            spmm_phase(g_bl, g_bl.phases[1], tabBb, 0.5, 0, bl_acc_mid, x0_bl,
                       bl_f1T, None, bl_acc_out)
            nc.gpsimd.collective_compute(
                "AllGather", mybir.AluOpType.bypass, ins=[bl_f1T[:]],
                outs=[bl_f1T_full[:]], replica_groups=RG)
            # P5: il-f1 user table; il-L2-alpha (item tiles)
            tabf1A = load_table_ag(il_f1T_full, IL_ROWS, 0, A_PAD, NTA // 2)
            spmm_phase(g_il, g_il.phases[0], tabf1A, 1.0 / 3, 1, il_acc_mid,
                       None, None, accT, il_acc_out)
            nc.gpsimd.collective_compute(
                "AllGather", mybir.AluOpType.bypass, ins=[accT[:]],
                outs=[accT_full[:]], replica_groups=RG)
            # P6: il-f1 item table; il-L2-beta (user tiles)
            tabf1B = load_table_ag(il_f1T_full, IL_ROWS, A_PAD, IL_ROWS,
                                   NTB_il // 2)
            spmm_phase(g_il, g_il.phases[1], tabf1B, 1.0 / 3, 1, il_acc_mid,
                       None, None, None, il_acc_out)
            # P7: bl-f1 user table; bl-L2-alpha
            tabg1A = load_table_ag(bl_f1T_full, BL_ROWS, 0, A_PAD, NTA // 2)
            spmm_phase(g_bl, g_bl.phases[0], tabg1A, 1.0 / 3, 1, bl_acc_mid,
                       None, None, None, bl_acc_out)
            # P8: bl-f1 bundle table; bl-L2-beta
            tabg1B = load_table_ag(bl_f1T_full, BL_ROWS, A_PAD, BL_ROWS,
                                   NTB_bl // 2)
            spmm_phase(g_bl, g_bl.phases[1], tabg1B, 1.0 / 3, 1, bl_acc_mid,
                       None, None, None, bl_acc_out)
            # P9: bi aggregation from item acc
            tabacc = load_table_ag(accT_full, IB_PAD, 0, IB_PAD, NTB_il // 2)
            spmm_phase(g_bi, g_bi.phases[0], tabacc, 1.0, None, None, None,
                       None, None, None)

    nc.compile()
    return nc


def kernel(users_feature, items_feature, bundles_feature,
           il_rows, il_cols, il_vals,
           bl_rows, bl_cols, bl_vals,
           bi_rows, bi_cols, bi_vals):
    global _compiled
    from concourse.bass_utils import run_bass_kernel_spmd

    xu = np.asarray(users_feature, np.float32)
    xi = np.asarray(items_feature, np.float32)
    xb = np.asarray(bundles_feature, np.float32)

    g_il, g_bl, g_bi = _prep(dict(
        il_rows=il_rows, il_cols=il_cols, il_vals=il_vals,
        bl_rows=bl_rows, bl_cols=bl_cols, bl_vals=bl_vals,
        bi_rows=bi_rows, bi_cols=bi_cols, bi_vals=bi_vals))

    if _compiled is None:
        _compiled = _build_program(g_il, g_bl, g_bi)
    nc = _compiled

    # host tables (transposed, dealt order, padded)
    def dealt_T(x, per, pad):
        n = x.shape[0]
        out = np.zeros((64, NC * pad, 1), np.float32)
        g = np.arange(n)
        out[:, (g % NC) * pad + g // NC, 0] = x.T
        return out

    xA_T = dealt_T(xu, A_PER, A_PAD)
    xBi_T = dealt_T(xi, IB_PER, IB_PAD)
    xBb_T = dealt_T(xb, BB_PER, BB_PAD)

    iota_np = np.tile(np.arange(128, dtype=np.float32),
                      (128, 1)).astype(BF16)
    ident_np = np.eye(128, dtype=np.float32)

    def x0_slices(xa, xbs, padA, padB):
        out = np.zeros((NC, padA + padB, D), np.float32)
        ga = np.arange(xa.shape[0])
        out[ga % NC, ga // NC] = xa
        gb = np.arange(xbs.shape[0])
        out[gb % NC, padA + gb // NC] = xbs
        return out

    x0_il = x0_slices(xu, xi, A_PAD, IB_PAD)
    x0_bl = x0_slices(xu, xb, A_PAD, BB_PAD)

    in_maps = []
    for c in range(NC):
        m = {"xA_T": xA_T, "xBi_T": xBi_T, "xBb_T": xBb_T,
             "x0_il": x0_il[c], "x0_bl": x0_bl[c],
             "iota_d": iota_np, "ident_d": ident_np}
        for g in (g_il, g_bl, g_bi):
            m[f"{g.name}_idx"] = g.idx16[c]
            m[f"{g.name}_rows"] = g.rows_f[c]
            m[f"{g.name}_vals"] = g.vals_f[c]
        in_maps.append(m)

    res = run_bass_kernel_spmd(nc, in_maps, core_ids=list(range(NC)))
    kernel.last_exec_ns = res.exec_time_ns

    il_acc = np.stack([res.results[c]["il_acc_out"] for c in range(NC)])
    bl_acc = np.stack([res.results[c]["bl_acc_out"] for c in range(NC)])
    bi_o = np.stack([res.results[c]["bi_out"] for c in range(NC)])

    gu = np.arange(U)
    gi = np.arange(I)
    gb = np.arange(B)
    il_users = il_acc[gu % NC, gu // NC]
    bl_users = bl_acc[gu % NC, gu // NC]
    il_bundles = bi_o[gb % NC, gb // NC]
    bl_bundles = bl_acc[gb % NC, A_PAD + gb // NC]
    return np.concatenate([il_users, bl_users, il_bundles, bl_bundles], 0)
